# revision 13
# baseline (speedup 1.0000x reference)
"""Trainium2 Bass kernel for fused causal multi-head attention
(qkv projection + causal softmax attention), B=2, T=4096, C=768, nH=12.

Sharding: 8 cores, core c -> batch b=c//4, head group g=c%4 (3 heads each).
Host transposes x[b] to xT [C,T] and column-reorders the weight stack.

v3 design (ACT-saturation + full projection/attention overlap):
  - QKV^T projection uses full-128-contraction matmuls into PSUM tiles that
    TIME-SHARE the lo-stream's attention PSUM tag (no extra banks); emission
    is interleaved with attention units gated on "projection step J done".
  - V^T -> V_aug via one DMA xbar transpose per (head, 512-token step):
    [64,512] -> [128,4,64] strided directly into v_aug; col 64 of each
    VST block pre-memset to 1.0 (softmax denominator ones row).
  - attention in S^T orientation, two interleaved streams on PE row-halves.
    Per 3-chunk group: S^T matmuls (causal column trim on diagonal chunks)
    -> EXP (ACT, no pre-mask) -> gpsimd affine_select zeroes masked P^T
    post-exp -> PV matmuls (1-group delayed). S->EXP has no DVE dependency.
  - normalize: per unit only [po copy + denom-row DMA] frees the PSUM bank;
    reciprocal batched over 4 units (reciprocal_approx_fast), gpsimd
    broadcast, multiply + output DMA deferred/staggered.
  PSUM: ps_lo/proj 3 + po_lo 1 + ps_hi 3 + po_hi 1 = 8 banks.
"""
import sys
sys.path.insert(0, '/opt/trn_rl_repo')
from collections import deque
import numpy as np

import concourse.bass as bass
import concourse.tile as tile
from concourse import bacc, mybir
from concourse import bass_utils

B, T, C, NH = 2, 4096, 768, 12
HD = 64
HPC = 3
NCORES = 8
NQ = T // 512
NKC = T // 128
GRP = 3
VST = 80   # v_aug block stride: 80*2B = 160B, 32B-aligned

BF = mybir.dt.bfloat16
CD = BF
F32 = mybir.dt.float32
AF = mybir.ActivationFunctionType
AL = mybir.AluOpType

RECIP_FAST = True
S_TRIM = True
PUMP_OPS = 14    # attention PE-ops pumped per projection sub-step
KEEPWARM = 1     # paced filler LDWEIGHTS per EXP group (HAM warm-keeping)

_CACHE = {}


def _build():
    if 'nc' in _CACHE:
        return _CACHE['nc']
    nc = bacc.Bacc("TRN2", target_bir_lowering=False, debug=False,
                   enable_asserts=True, num_devices=NCORES)
    xT_d = nc.dram_tensor("xT", [C, T], CD, kind="ExternalInput").ap()
    w_d = nc.dram_tensor("w", [C, 576], CD, kind="ExternalInput").ap()
    b_d = nc.dram_tensor("b", [128, 5], F32, kind="ExternalInput").ap()
    out_d = nc.dram_tensor("out", [HPC * HD, T], F32, kind="ExternalOutput").ap()

    # head slot map: (q_tile, q_lo, k_tile, k_lo, v_tile, v_lo)
    # tile0=[Q0;Q1] tile1=[K0;K1] tile2=[Q2;V0] tile3=[K2;V1] tile4=[V2]
    SLOT = [
        (0, 0, 1, 0, 2, 64),
        (0, 64, 1, 64, 3, 64),
        (2, 0, 3, 0, 4, 0),
    ]

    with tile.TileContext(nc) as tc:
        with (
            tc.tile_pool(name="const", bufs=1) as cpool,
            tc.tile_pool(name="persist", bufs=1) as sb,
            tc.tile_pool(name="wsb", bufs=1) as wpool,
            tc.tile_pool(name="xn", bufs=12) as xpool,
            tc.tile_pool(name="ps_lo", bufs=1, space="PSUM") as pslo,
            tc.tile_pool(name="ps_hi", bufs=1, space="PSUM") as pshi,
            tc.tile_pool(name="pT", bufs=3) as ptp,
            tc.tile_pool(name="nrm", bufs=2) as nrm,
        ):
            bias_sb = cpool.tile([128, 5], F32)
            nc.sync.dma_start(bias_sb[:], b_d[:])

            qkv_sb = [sb.tile([128, T], CD, name=f"qkv{m}") for m in range(4)]
            qkv_sb.append(sb.tile([64, T], CD, name="qkv4"))
            # duplicates of h2's Q^T/K^T at partitions 64-127 (for the HI stream)
            qdup = sb.tile([128, T], CD, name="qdup")
            kdup = sb.tile([128, T], CD, name="kdup")
            v_aug = [sb.tile([128, NKC * VST], CD, name=f"vaug{h}")
                     for h in range(HPC)]
            # col 64 of every VST block must be 1.0 (denominator ones row);
            # cols 0-63 get overwritten by the V^T xbar transposes.
            v_aug3 = []
            for h in range(HPC):
                nc.vector.memset(v_aug[h][:], 1.0)
                v_aug3.append(v_aug[h][:].rearrange("p (n s) -> p n s", s=VST))

            w_sb = [wpool.tile([128, 576], CD, name=f"w{k}") for k in range(6)]
            for k in range(6):
                nc.sync.dma_start(w_sb[k][:], w_d[128 * k:128 * (k + 1), :])

            # ---------- projection generator (time-shares lo PSUM tag) ----
            def proj_mblock(pj, col, n, m):
                mw = 128 if m < 4 else 64
                for k in range(6):
                    nc.tensor.matmul(
                        pj[:mw, 512 * col:512 * col + 512],
                        lhsT=w_sb[k][:, 128 * m:128 * m + mw],
                        rhs=xn_cur[k][:], start=(k == 0), stop=(k == 5))
                nc.vector.tensor_scalar(
                    out=qkv_sb[m][:mw, 512 * n:512 * (n + 1)],
                    in0=pj[:mw, 512 * col:512 * col + 512],
                    scalar1=bias_sb[:mw, m:m + 1], scalar2=None, op0=AL.add)
                if m >= 2:
                    h = m - 2
                    vt, vlo = SLOT[h][4], SLOT[h][5]
                    nc.sync.dma_start_transpose(
                        v_aug3[h][:, 4 * n:4 * n + 4, 0:64],
                        qkv_sb[vt][vlo:vlo + 64, 512 * n:512 * (n + 1)])
                    if h == 0:
                        nc.sync.dma_start(
                            qdup[64:128, 512 * n:512 * (n + 1)],
                            qkv_sb[2][0:64, 512 * n:512 * (n + 1)])
                    elif h == 1:
                        nc.sync.dma_start(
                            kdup[64:128, 512 * n:512 * (n + 1)],
                            qkv_sb[3][0:64, 512 * n:512 * (n + 1)])

            xn_cur = None

            def proj_gen():
                nonlocal xn_cur
                for n in range(NQ):
                    xn_cur = []
                    for k in range(6):
                        t = xpool.tile([128, 512], CD, tag="xn",
                                       name=f"xn{n}_{k}")
                        nc.sync.dma_start(
                            t[:],
                            xT_d[128 * k:128 * (k + 1), 512 * n:512 * (n + 1)])
                        xn_cur.append(t)
                    pjA = pslo.tile([128, GRP * 512], F32, tag="pslo",
                                    name=f"pjA{n}", bufs=1)
                    for m in range(3):
                        proj_mblock(pjA, m, n, m)
                    yield ('mid', n)
                    pjB = pshi.tile([128, GRP * 512], F32, tag="pshi",
                                    name=f"pjB{n}", bufs=1)
                    for m in range(3, 5):
                        proj_mblock(pjB, m - 3, n, m)
                    yield ('done', n)

            # ---------- attention machinery ----------
            class SState:
                def __init__(self, side):
                    self.side = side
                    self.slot = 0
                    self.batch_id = 0
                    self.den = None
                    self.rcp_by_batch = {}
                    self.finishq = deque()

            def flush_recip(st):
                if st.slot == 0:
                    return
                rcp = nrm.tile([4, 512], F32, tag=f"rcp{st.side}",
                               name=f"rcp_{st.side}_{st.batch_id}", bufs=2)
                if RECIP_FAST:
                    nc.vector.reciprocal_approx_fast(
                        out=rcp[0:st.slot, :], in_=st.den[0:st.slot, :])
                else:
                    nc.vector.reciprocal(rcp[0:st.slot, :],
                                         st.den[0:st.slot, :])
                st.rcp_by_batch[st.batch_id] = rcp
                st.batch_id += 1
                st.slot = 0
                st.den = None

            def maybe_finish(st, limit):
                done = 0
                while st.finishq and done < limit:
                    h, J, uo, bid, slot = st.finishq[0]
                    if bid not in st.rcp_by_batch:
                        break
                    st.finishq.popleft()
                    rcp = st.rcp_by_batch[bid]
                    rcp0 = nrm.tile([1, 512], F32, tag=f"rcp0{st.side}",
                                    name=f"rc0_{st.side}_{h}_{J}", bufs=2)
                    nc.sync.dma_start(rcp0[:], rcp[slot:slot + 1, :])
                    rb = nrm.tile([64, 512], F32, tag=f"rb{st.side}",
                                  name=f"rb_{st.side}_{h}_{J}", bufs=2)
                    nc.gpsimd.partition_broadcast(rb[:], rcp0[:])
                    otn = nrm.tile([64, 512], F32, tag=f"otn{st.side}",
                                   name=f"ot_{st.side}_{h}_{J}", bufs=2)
                    nc.vector.tensor_tensor(out=otn[:], in0=uo[0:64, :],
                                            in1=rb[:], op=AL.mult)
                    nc.sync.dma_start(
                        out_d[HD * h:HD * (h + 1), 512 * J:512 * (J + 1)],
                        otn[:])
                    done += 1

            def emit_pv(pend, po, h, J):
                pg0, pg1, ppT = pend
                nK = 4 * (J + 1)
                for kc in range(pg0, pg1):
                    jj = kc - pg0
                    nc.tensor.matmul(
                        po[:], lhsT=v_aug[h][:, VST * kc:VST * kc + 65],
                        rhs=ppT[:, 512 * jj:512 * (jj + 1)],
                        start=(kc == 0), stop=(kc == nK - 1))
                    yield

            def unit(side, h, J, qs_ap, k_tile, k_lo, pspool, potag, st):
                nK = 4 * (J + 1)
                po = pspool.tile([65, 512], F32, tag=potag,
                                 name=f"po_{side}_{h}_{J}", bufs=1)
                pending = None
                for g0 in range(0, nK, GRP):
                    g1 = min(g0 + GRP, nK)
                    wid = 512 * (g1 - g0)
                    ps_s = pspool.tile([128, GRP * 512], F32,
                                       tag=f"ps{side}",
                                       name=f"ps_{side}_{h}_{J}_{g0}",
                                       bufs=1)
                    for kc in range(g0, g1):
                        jj = kc - g0
                        d = kc - 4 * J
                        c0 = 128 * d if (S_TRIM and d > 0) else 0
                        nc.tensor.matmul(
                            ps_s[:, 512 * jj + c0:512 * (jj + 1)],
                            lhsT=k_tile[k_lo:k_lo + 64,
                                        128 * kc:128 * (kc + 1)],
                            rhs=qs_ap[:, c0:512], start=True, stop=True)
                        yield
                    pT = ptp.tile([128, GRP * 512], CD, tag=f"pT{side}",
                                  name=f"pT_{side}_{h}_{J}_{g0}", bufs=3)
                    nc.scalar.activation(pT[:, :wid], ps_s[:, :wid],
                                         AF.Exp, scale=0.125)
                    for kc in range(g0, g1):
                        d = kc - 4 * J
                        if d >= 0:
                            jj = kc - g0
                            nc.gpsimd.affine_select(
                                out=pT[:, 512 * jj:512 * (jj + 1)],
                                in_=pT[:, 512 * jj:512 * (jj + 1)],
                                compare_op=AL.is_ge, fill=0.0,
                                base=-128 * d, channel_multiplier=-1,
                                pattern=[[1, 512]])
                    # paced PE activity during the ACT window: a throwaway
                    # matmul reading pT (post-exp/mask) and scribbling on the
                    # already-consumed ps_s buffer. It executes right as EXP
                    # finishes, keeping the HAM activity monitor from
                    # re-throttling the PE clock during ACT-bound stretches.
                    # The scribbled region is either rewritten (start=True) by
                    # the next group's S matmuls or lands in the causally
                    # masked zone that affine_select zeroes post-exp.
                    for _ in range(KEEPWARM):
                        nc.tensor.matmul(
                            ps_s[0:64, 0:128], lhsT=pT[0:64, 0:64],
                            rhs=pT[0:64, 0:128], start=True, stop=True)
                    if pending is not None:
                        yield from emit_pv(pending, po, h, J)
                    pending = (g0, g1, pT)
                    yield
                yield from emit_pv(pending, po, h, J)
                # normalize prologue: free the po bank quickly
                # (rows 0-63 = unnormalized out, row 64 = denominator)
                uo = nrm.tile([65, 512], F32, tag=f"uo{side}",
                              name=f"uo_{side}_{h}_{J}", bufs=6)
                nc.vector.tensor_copy(uo[:], po[:])
                if st.slot == 0:
                    st.den = nrm.tile([4, 512], F32, tag=f"den{side}",
                                      name=f"den_{side}_{st.batch_id}",
                                      bufs=2)
                nc.sync.dma_start(st.den[st.slot:st.slot + 1, :],
                                  uo[64:65, :])
                st.finishq.append((h, J, uo, st.batch_id, st.slot))
                st.slot += 1
                if st.slot == 4:
                    flush_recip(st)
                maybe_finish(st, limit=1)

            def stream(side, units, pspool, potag, st):
                for h, J, qs_ap, k_tile, k_lo in units:
                    yield ('gate', J)
                    yield from unit(side, h, J, qs_ap, k_tile, k_lo,
                                    pspool, potag, st)
                flush_recip(st)
                maybe_finish(st, limit=len(st.finishq))

            def qs(tile_idx, lo, J):
                return qkv_sb[tile_idx][lo:lo + 64, 512 * J:512 * (J + 1)]

            h2_lo = (3, 5, 7)
            h2_hi = tuple(J for J in range(NQ) if J not in h2_lo)
            lo_units = [(0, J, qs(0, 0, J), qkv_sb[1], 0) for J in range(NQ)]
            lo_units += [(2, J, qs(2, 0, J), qkv_sb[3], 0) for J in h2_lo]
            hi_units = [(1, J, qs(0, 64, J), qkv_sb[1], 64) for J in range(NQ)]
            hi_units += [(2, J, qdup[64:128, 512 * J:512 * (J + 1)], kdup, 64)
                         for J in h2_hi]
            lo_units.sort(key=lambda u: (u[1], u[0]))
            hi_units.sort(key=lambda u: (u[1], u[0]))

            class Pumped:
                def __init__(self, gen):
                    self.gen = gen
                    self.parked = None
                    self.alive = True

            streams = [
                Pumped(stream("lo", lo_units, pslo, "polo", SState("lo"))),
                Pumped(stream("hi", hi_units, pshi, "pohi", SState("hi"))),
            ]

            def pump(allowed, max_ops):
                ops = 0
                while ops < max_ops:
                    progress = False
                    for s in streams:
                        if not s.alive:
                            continue
                        if s.parked is not None and s.parked > allowed:
                            continue
                        s.parked = None
                        try:
                            y = next(s.gen)
                        except StopIteration:
                            s.alive = False
                            continue
                        if isinstance(y, tuple) and y[0] == 'gate':
                            s.parked = y[1]
                            if s.parked <= allowed:
                                s.parked = None
                                progress = True
                            continue
                        ops += 1
                        progress = True
                    if not progress:
                        break

            # gate only on fully-emitted projection steps: Tile dependency
            # tracking is emission-order-based, so consumers must be emitted
            # after their producers (e.g. kdup chunk DMAs land in the B half).
            allowed = -1
            for kind, n in proj_gen():
                if kind == 'done':
                    allowed = n
                pump(allowed, PUMP_OPS)
            pump(10 ** 9, 10 ** 9)

    nc.compile()
    _CACHE['nc'] = nc
    return nc


def _prep_inputs(x, w_qkv, b_qkv):
    """Host-side sharding: per-core xT, column-reordered weight stack, bias."""
    import ml_dtypes
    cdt = ml_dtypes.bfloat16
    x = np.asarray(x, dtype=np.float32)
    w_qkv = np.asarray(w_qkv, dtype=np.float32)
    b_qkv = np.asarray(b_qkv, dtype=np.float32)
    xTs = [np.ascontiguousarray(x[b].T).astype(cdt) for b in range(B)]
    in_maps = []
    for c in range(NCORES):
        b_idx, g = c // 4, c % 4
        H = [3 * g, 3 * g + 1, 3 * g + 2]
        q = lambda h: np.arange(64 * h, 64 * (h + 1))
        k = lambda h: np.arange(C + 64 * h, C + 64 * (h + 1))
        v = lambda h: np.arange(2 * C + 64 * h, 2 * C + 64 * (h + 1))
        cols = np.concatenate([
            q(H[0]), q(H[1]),
            k(H[0]), k(H[1]),
            q(H[2]), v(H[0]),
            k(H[2]), v(H[1]),
            v(H[2]),
        ])
        w_stack = np.ascontiguousarray(w_qkv[:, cols]).astype(cdt)
        b_stack = b_qkv[cols]
        bias_pad = np.zeros((128, 5), dtype=np.float32)
        for m in range(4):
            bias_pad[:, m] = b_stack[128 * m:128 * (m + 1)]
        bias_pad[:64, 4] = b_stack[512:576]
        in_maps.append({"xT": xTs[b_idx], "w": w_stack, "b": bias_pad})
    return in_maps


def _run(x, w_qkv, b_qkv, n_head, **run_kwargs):
    assert int(n_head) == NH and x.shape == (B, T, C)
    nc = _build()
    in_maps = _prep_inputs(x, w_qkv, b_qkv)
    res = bass_utils.run_bass_kernel_spmd(
        nc, in_maps, core_ids=list(range(NCORES)), **run_kwargs)
    out = np.empty((B, T, C), dtype=np.float32)
    for c in range(NCORES):
        b_idx, g = c // 4, c % 4
        out[b_idx, :, 192 * g:192 * (g + 1)] = res.results[c]["out"].T
    return out, res


def kernel(x, w_qkv, b_qkv, n_head):
    return _run(x, w_qkv, b_qkv, n_head)[0]


# revision 14
# speedup vs baseline: 1.5154x; 1.5154x over previous
"""Trainium2 Bass kernel for fused causal multi-head attention
(qkv projection + causal softmax attention), B=2, T=4096, C=768, nH=12.

Sharding: 8 cores, core c -> batch b=c//4, head group g=c%4 (3 heads each).
Host transposes x[b] to xT [C,T] and column-reorders the weight stack.

v3 design (ACT-saturation + full projection/attention overlap):
  - QKV^T projection uses full-128-contraction matmuls into PSUM tiles that
    TIME-SHARE the lo-stream's attention PSUM tag (no extra banks); emission
    is interleaved with attention units gated on "projection step J done".
  - V^T -> V_aug via one DMA xbar transpose per (head, 512-token step):
    [64,512] -> [128,4,64] strided directly into v_aug; col 64 of each
    VST block pre-memset to 1.0 (softmax denominator ones row).
  - attention in S^T orientation, two interleaved streams on PE row-halves.
    Per 3-chunk group: S^T matmuls (causal column trim on diagonal chunks)
    -> EXP (ACT, no pre-mask) -> gpsimd affine_select zeroes masked P^T
    post-exp -> PV matmuls (1-group delayed). S->EXP has no DVE dependency.
  - normalize: per unit only [po copy + denom-row DMA] frees the PSUM bank;
    reciprocal batched over 4 units (reciprocal_approx_fast), gpsimd
    broadcast, multiply + output DMA deferred/staggered.
  PSUM: ps_lo/proj 3 + po_lo 1 + ps_hi 3 + po_hi 1 = 8 banks.
"""
import sys
sys.path.insert(0, '/opt/trn_rl_repo')
from collections import deque
import numpy as np

import concourse.bass as bass
import concourse.tile as tile
from concourse import bacc, mybir
from concourse import bass_utils

B, T, C, NH = 2, 4096, 768, 12
HD = 64
HPC = 3
NCORES = 8
NQ = T // 512
NKC = T // 128
GRP = 3
VST = 80   # v_aug block stride: 80*2B = 160B, 32B-aligned

BF = mybir.dt.bfloat16
CD = BF
F32 = mybir.dt.float32
AF = mybir.ActivationFunctionType
AL = mybir.AluOpType

RECIP_FAST = True
S_TRIM = True
PUMP_OPS = 14    # attention PE-ops pumped per projection sub-step
KEEPWARM = 1     # paced filler LDWEIGHTS per EXP group (HAM warm-keeping)

_CACHE = {}


def _build():
    if 'nc' in _CACHE:
        return _CACHE['nc']
    nc = bacc.Bacc("TRN2", target_bir_lowering=False, debug=False,
                   enable_asserts=True, num_devices=NCORES)
    xT_d = nc.dram_tensor("xT", [C, T], CD, kind="ExternalInput").ap()
    w_d = nc.dram_tensor("w", [C, 576], CD, kind="ExternalInput").ap()
    b_d = nc.dram_tensor("b", [128, 5], F32, kind="ExternalInput").ap()
    out_d = nc.dram_tensor("out", [HPC * HD, T], F32, kind="ExternalOutput").ap()

    # head slot map: (q_tile, q_lo, k_tile, k_lo, v_tile, v_lo)
    # tile0=[Q0;Q1] tile1=[K0;K1] tile2=[Q2;V0] tile3=[K2;V1] tile4=[V2]
    SLOT = [
        (0, 0, 1, 0, 2, 64),
        (0, 64, 1, 64, 3, 64),
        (2, 0, 3, 0, 4, 0),
    ]

    with tile.TileContext(nc) as tc:
        with (
            tc.tile_pool(name="const", bufs=1) as cpool,
            tc.tile_pool(name="persist", bufs=1) as sb,
            tc.tile_pool(name="wsb", bufs=1) as wpool,
            tc.tile_pool(name="xn", bufs=12) as xpool,
            tc.tile_pool(name="ps_lo", bufs=1, space="PSUM") as pslo,
            tc.tile_pool(name="ps_hi", bufs=1, space="PSUM") as pshi,
            tc.tile_pool(name="pT", bufs=3) as ptp,
            tc.tile_pool(name="nrm", bufs=2) as nrm,
        ):
            bias_sb = cpool.tile([128, 5], F32)
            nc.sync.dma_start(bias_sb[:], b_d[:])

            qkv_sb = [sb.tile([128, T], CD, name=f"qkv{m}") for m in range(4)]
            qkv_sb.append(sb.tile([64, T], CD, name="qkv4"))
            # duplicates of h2's Q^T/K^T at partitions 64-127 (for the HI stream)
            qdup = sb.tile([128, T], CD, name="qdup")
            kdup = sb.tile([128, T], CD, name="kdup")
            v_aug = [sb.tile([128, NKC * VST], CD, name=f"vaug{h}")
                     for h in range(HPC)]
            # col 64 of every VST block must be 1.0 (denominator ones row);
            # cols 0-63 get overwritten by the V^T xbar transposes.
            v_aug3 = []
            for h in range(HPC):
                nc.vector.memset(v_aug[h][:], 1.0)
                v_aug3.append(v_aug[h][:].rearrange("p (n s) -> p n s", s=VST))

            w_sb = [wpool.tile([128, 576], CD, name=f"w{k}") for k in range(6)]
            for k in range(6):
                nc.sync.dma_start(w_sb[k][:], w_d[128 * k:128 * (k + 1), :])

            # ---------- projection generator (time-shares stream PSUM tags) --
            def proj_mms(pj, col, m):
                mw = 128 if m < 4 else 64
                for k in range(6):
                    nc.tensor.matmul(
                        pj[:mw, 512 * col:512 * col + 512],
                        lhsT=w_sb[k][:, 128 * m:128 * m + mw],
                        rhs=xn_cur[k][:], start=(k == 0), stop=(k == 5))

            def proj_scatter(stg, col, n, m):
                # bias-add + scatter from bf16 staging (off the PSUM chain)
                mw = 128 if m < 4 else 64
                nc.vector.tensor_scalar(
                    out=qkv_sb[m][:mw, 512 * n:512 * (n + 1)],
                    in0=stg[:mw, 512 * col:512 * col + 512],
                    scalar1=bias_sb[:mw, m:m + 1], scalar2=None, op0=AL.add)
                if m >= 2:
                    h = m - 2
                    vt, vlo = SLOT[h][4], SLOT[h][5]
                    nc.sync.dma_start_transpose(
                        v_aug3[h][:, 4 * n:4 * n + 4, 0:64],
                        qkv_sb[vt][vlo:vlo + 64, 512 * n:512 * (n + 1)])
                    if h == 0:
                        nc.sync.dma_start(
                            qdup[64:128, 512 * n:512 * (n + 1)],
                            qkv_sb[2][0:64, 512 * n:512 * (n + 1)])
                    elif h == 1:
                        nc.sync.dma_start(
                            kdup[64:128, 512 * n:512 * (n + 1)],
                            qkv_sb[3][0:64, 512 * n:512 * (n + 1)])

            xn_cur = None

            def proj_gen():
                nonlocal xn_cur
                for n in range(NQ):
                    xn_cur = []
                    for k in range(6):
                        t = xpool.tile([128, 512], CD, tag="xn",
                                       name=f"xn{n}_{k}")
                        nc.sync.dma_start(
                            t[:],
                            xT_d[128 * k:128 * (k + 1), 512 * n:512 * (n + 1)])
                        xn_cur.append(t)
                    pjA = pslo.tile([128, GRP * 512], F32, tag="pslo",
                                    name=f"pjA{n}", bufs=1)
                    for m in range(3):
                        proj_mms(pjA, m, m)
                    stgA = xpool.tile([128, GRP * 512], CD, tag="stg",
                                      name=f"stgA{n}", bufs=2)
                    nc.vector.tensor_copy(stgA[:], pjA[:])
                    for m in range(3):
                        proj_scatter(stgA, m, n, m)
                    yield ('mid', n)
                    pjB = pshi.tile([128, GRP * 512], F32, tag="pshi",
                                    name=f"pjB{n}", bufs=1)
                    for m in range(3, 5):
                        proj_mms(pjB, m - 3, m)
                    stgB = xpool.tile([128, GRP * 512], CD, tag="stg",
                                      name=f"stgB{n}", bufs=2)
                    nc.vector.tensor_copy(stgB[:, 0:1024], pjB[:, 0:1024])
                    for m in range(3, 5):
                        proj_scatter(stgB, m - 3, n, m)
                    yield ('done', n)

            # ---------- attention machinery ----------
            class SState:
                def __init__(self, side):
                    self.side = side
                    self.slot = 0
                    self.batch_id = 0
                    self.den = None
                    self.rcp_by_batch = {}
                    self.finishq = deque()

            def flush_recip(st):
                if st.slot == 0:
                    return
                rcp = nrm.tile([4, 512], F32, tag=f"rcp{st.side}",
                               name=f"rcp_{st.side}_{st.batch_id}", bufs=2)
                if RECIP_FAST:
                    nc.vector.reciprocal_approx_fast(
                        out=rcp[0:st.slot, :], in_=st.den[0:st.slot, :])
                else:
                    nc.vector.reciprocal(rcp[0:st.slot, :],
                                         st.den[0:st.slot, :])
                st.rcp_by_batch[st.batch_id] = rcp
                st.batch_id += 1
                st.slot = 0
                st.den = None

            def maybe_finish(st, limit):
                done = 0
                while st.finishq and done < limit:
                    h, J, uo, bid, slot = st.finishq[0]
                    if bid not in st.rcp_by_batch:
                        break
                    st.finishq.popleft()
                    rcp = st.rcp_by_batch[bid]
                    rcp0 = nrm.tile([1, 512], F32, tag=f"rcp0{st.side}",
                                    name=f"rc0_{st.side}_{h}_{J}", bufs=2)
                    nc.sync.dma_start(rcp0[:], rcp[slot:slot + 1, :])
                    rb = nrm.tile([64, 512], F32, tag=f"rb{st.side}",
                                  name=f"rb_{st.side}_{h}_{J}", bufs=2)
                    nc.gpsimd.partition_broadcast(rb[:], rcp0[:])
                    otn = nrm.tile([64, 512], F32, tag=f"otn{st.side}",
                                   name=f"ot_{st.side}_{h}_{J}", bufs=2)
                    nc.vector.tensor_tensor(out=otn[:], in0=uo[0:64, :],
                                            in1=rb[:], op=AL.mult)
                    nc.sync.dma_start(
                        out_d[HD * h:HD * (h + 1), 512 * J:512 * (J + 1)],
                        otn[:])
                    done += 1

            def emit_pv(pend, po, h, J):
                pg0, pg1, ppT = pend
                nK = 4 * (J + 1)
                for kc in range(pg0, pg1):
                    jj = kc - pg0
                    nc.tensor.matmul(
                        po[:], lhsT=v_aug[h][:, VST * kc:VST * kc + 65],
                        rhs=ppT[:, 512 * jj:512 * (jj + 1)],
                        start=(kc == 0), stop=(kc == nK - 1))
                    yield

            def unit(side, h, J, qs_ap, k_tile, k_lo, pspool, potag, st):
                nK = 4 * (J + 1)
                po = pspool.tile([65, 512], F32, tag=potag,
                                 name=f"po_{side}_{h}_{J}", bufs=1)
                pending = None
                for g0 in range(0, nK, GRP):
                    g1 = min(g0 + GRP, nK)
                    wid = 512 * (g1 - g0)
                    ps_s = pspool.tile([128, GRP * 512], F32,
                                       tag=f"ps{side}",
                                       name=f"ps_{side}_{h}_{J}_{g0}",
                                       bufs=1)
                    for kc in range(g0, g1):
                        jj = kc - g0
                        d = kc - 4 * J
                        c0 = 128 * d if (S_TRIM and d > 0) else 0
                        nc.tensor.matmul(
                            ps_s[:, 512 * jj + c0:512 * (jj + 1)],
                            lhsT=k_tile[k_lo:k_lo + 64,
                                        128 * kc:128 * (kc + 1)],
                            rhs=qs_ap[:, c0:512], start=True, stop=True)
                        yield
                    pT = ptp.tile([128, GRP * 512], CD, tag=f"pT{side}",
                                  name=f"pT_{side}_{h}_{J}_{g0}", bufs=3)
                    nc.scalar.activation(pT[:, :wid], ps_s[:, :wid],
                                         AF.Exp, scale=0.125)
                    for kc in range(g0, g1):
                        d = kc - 4 * J
                        if d >= 0:
                            jj = kc - g0
                            nc.gpsimd.affine_select(
                                out=pT[:, 512 * jj:512 * (jj + 1)],
                                in_=pT[:, 512 * jj:512 * (jj + 1)],
                                compare_op=AL.is_ge, fill=0.0,
                                base=-128 * d, channel_multiplier=-1,
                                pattern=[[1, 512]])
                    # paced PE activity during the ACT window: a throwaway
                    # matmul reading pT (post-exp/mask) and scribbling on the
                    # already-consumed ps_s buffer. It executes right as EXP
                    # finishes, keeping the HAM activity monitor from
                    # re-throttling the PE clock during ACT-bound stretches.
                    # The scribbled region is either rewritten (start=True) by
                    # the next group's S matmuls or lands in the causally
                    # masked zone that affine_select zeroes post-exp.
                    for _ in range(KEEPWARM):
                        nc.tensor.matmul(
                            ps_s[0:64, 0:128], lhsT=pT[0:64, 0:64],
                            rhs=pT[0:64, 0:128], start=True, stop=True)
                    if pending is not None:
                        yield from emit_pv(pending, po, h, J)
                    pending = (g0, g1, pT)
                    yield
                yield from emit_pv(pending, po, h, J)
                # normalize prologue: free the po bank quickly
                # (rows 0-63 = unnormalized out, row 64 = denominator)
                uo = nrm.tile([65, 512], F32, tag=f"uo{side}",
                              name=f"uo_{side}_{h}_{J}", bufs=6)
                nc.vector.tensor_copy(uo[:], po[:])
                if st.slot == 0:
                    st.den = nrm.tile([4, 512], F32, tag=f"den{side}",
                                      name=f"den_{side}_{st.batch_id}",
                                      bufs=2)
                nc.sync.dma_start(st.den[st.slot:st.slot + 1, :],
                                  uo[64:65, :])
                st.finishq.append((h, J, uo, st.batch_id, st.slot))
                st.slot += 1
                if st.slot == 4:
                    flush_recip(st)
                maybe_finish(st, limit=1)

            def stream(side, units, pspool, potag, st):
                for h, J, qs_ap, k_tile, k_lo in units:
                    yield ('gate', J)
                    yield from unit(side, h, J, qs_ap, k_tile, k_lo,
                                    pspool, potag, st)
                flush_recip(st)
                maybe_finish(st, limit=len(st.finishq))

            def qs(tile_idx, lo, J):
                return qkv_sb[tile_idx][lo:lo + 64, 512 * J:512 * (J + 1)]

            h2_lo = (3, 5, 7)
            h2_hi = tuple(J for J in range(NQ) if J not in h2_lo)
            lo_units = [(0, J, qs(0, 0, J), qkv_sb[1], 0) for J in range(NQ)]
            lo_units += [(2, J, qs(2, 0, J), qkv_sb[3], 0) for J in h2_lo]
            hi_units = [(1, J, qs(0, 64, J), qkv_sb[1], 64) for J in range(NQ)]
            hi_units += [(2, J, qdup[64:128, 512 * J:512 * (J + 1)], kdup, 64)
                         for J in h2_hi]
            lo_units.sort(key=lambda u: (u[1], u[0]))
            hi_units.sort(key=lambda u: (u[1], u[0]))

            class Pumped:
                def __init__(self, gen):
                    self.gen = gen
                    self.parked = None
                    self.alive = True

            streams = [
                Pumped(stream("lo", lo_units, pslo, "polo", SState("lo"))),
                Pumped(stream("hi", hi_units, pshi, "pohi", SState("hi"))),
            ]

            def pump(allowed, max_ops):
                ops = 0
                while ops < max_ops:
                    progress = False
                    for s in streams:
                        if not s.alive:
                            continue
                        if s.parked is not None and s.parked > allowed:
                            continue
                        s.parked = None
                        try:
                            y = next(s.gen)
                        except StopIteration:
                            s.alive = False
                            continue
                        if isinstance(y, tuple) and y[0] == 'gate':
                            s.parked = y[1]
                            if s.parked <= allowed:
                                s.parked = None
                                progress = True
                            continue
                        ops += 1
                        progress = True
                    if not progress:
                        break

            # gate only on fully-emitted projection steps: Tile dependency
            # tracking is emission-order-based, so consumers must be emitted
            # after their producers (e.g. kdup chunk DMAs land in the B half).
            allowed = -1
            for kind, n in proj_gen():
                if kind == 'done':
                    allowed = n
                pump(allowed, PUMP_OPS)
            pump(10 ** 9, 10 ** 9)

    nc.compile()
    _CACHE['nc'] = nc
    return nc


def _prep_inputs(x, w_qkv, b_qkv):
    """Host-side sharding: per-core xT, column-reordered weight stack, bias."""
    import ml_dtypes
    cdt = ml_dtypes.bfloat16
    x = np.asarray(x, dtype=np.float32)
    w_qkv = np.asarray(w_qkv, dtype=np.float32)
    b_qkv = np.asarray(b_qkv, dtype=np.float32)
    xTs = [np.ascontiguousarray(x[b].T).astype(cdt) for b in range(B)]
    in_maps = []
    for c in range(NCORES):
        b_idx, g = c // 4, c % 4
        H = [3 * g, 3 * g + 1, 3 * g + 2]
        q = lambda h: np.arange(64 * h, 64 * (h + 1))
        k = lambda h: np.arange(C + 64 * h, C + 64 * (h + 1))
        v = lambda h: np.arange(2 * C + 64 * h, 2 * C + 64 * (h + 1))
        cols = np.concatenate([
            q(H[0]), q(H[1]),
            k(H[0]), k(H[1]),
            q(H[2]), v(H[0]),
            k(H[2]), v(H[1]),
            v(H[2]),
        ])
        w_stack = np.ascontiguousarray(w_qkv[:, cols]).astype(cdt)
        b_stack = b_qkv[cols]
        bias_pad = np.zeros((128, 5), dtype=np.float32)
        for m in range(4):
            bias_pad[:, m] = b_stack[128 * m:128 * (m + 1)]
        bias_pad[:64, 4] = b_stack[512:576]
        in_maps.append({"xT": xTs[b_idx], "w": w_stack, "b": bias_pad})
    return in_maps


def _run(x, w_qkv, b_qkv, n_head, **run_kwargs):
    assert int(n_head) == NH and x.shape == (B, T, C)
    nc = _build()
    in_maps = _prep_inputs(x, w_qkv, b_qkv)
    res = bass_utils.run_bass_kernel_spmd(
        nc, in_maps, core_ids=list(range(NCORES)), **run_kwargs)
    out = np.empty((B, T, C), dtype=np.float32)
    for c in range(NCORES):
        b_idx, g = c // 4, c % 4
        out[b_idx, :, 192 * g:192 * (g + 1)] = res.results[c]["out"].T
    return out, res


def kernel(x, w_qkv, b_qkv, n_head):
    return _run(x, w_qkv, b_qkv, n_head)[0]


# revision 15
# speedup vs baseline: 1.5161x; 1.0004x over previous
"""Trainium2 Bass kernel for fused causal multi-head attention
(qkv projection + causal softmax attention), B=2, T=4096, C=768, nH=12.

Sharding: 8 cores, core c -> batch b=c//4, head group g=c%4 (3 heads each).
Host transposes x[b] to xT [C,T] and column-reorders the weight stack.

v3 design (ACT-saturation + full projection/attention overlap):
  - QKV^T projection uses full-128-contraction matmuls into PSUM tiles that
    TIME-SHARE the lo-stream's attention PSUM tag (no extra banks); emission
    is interleaved with attention units gated on "projection step J done".
  - V^T -> V_aug via one DMA xbar transpose per (head, 512-token step):
    [64,512] -> [128,4,64] strided directly into v_aug; col 64 of each
    VST block pre-memset to 1.0 (softmax denominator ones row).
  - attention in S^T orientation, two interleaved streams on PE row-halves.
    Per 3-chunk group: S^T matmuls (causal column trim on diagonal chunks)
    -> EXP (ACT, no pre-mask) -> gpsimd affine_select zeroes masked P^T
    post-exp -> PV matmuls (1-group delayed). S->EXP has no DVE dependency.
  - normalize: per unit only [po copy + denom-row DMA] frees the PSUM bank;
    reciprocal batched over 4 units (reciprocal_approx_fast), gpsimd
    broadcast, multiply + output DMA deferred/staggered.
  PSUM: ps_lo/proj 3 + po_lo 1 + ps_hi 3 + po_hi 1 = 8 banks.
"""
import sys
sys.path.insert(0, '/opt/trn_rl_repo')
from collections import deque
import numpy as np

import concourse.bass as bass
import concourse.tile as tile
from concourse import bacc, mybir
from concourse import bass_utils

B, T, C, NH = 2, 4096, 768, 12
HD = 64
HPC = 3
NCORES = 8
NQ = T // 512
NKC = T // 128
GRP = 3
VST = 80   # v_aug block stride: 80*2B = 160B, 32B-aligned

BF = mybir.dt.bfloat16
CD = BF
F32 = mybir.dt.float32
AF = mybir.ActivationFunctionType
AL = mybir.AluOpType

RECIP_FAST = True
S_TRIM = True
PUMP_OPS = 40    # attention PE-ops pumped per projection sub-step
KEEPWARM = 1     # paced filler LDWEIGHTS per EXP group (HAM warm-keeping)

_CACHE = {}


def _build():
    if 'nc' in _CACHE:
        return _CACHE['nc']
    nc = bacc.Bacc("TRN2", target_bir_lowering=False, debug=False,
                   enable_asserts=True, num_devices=NCORES)
    xT_d = nc.dram_tensor("xT", [C, T], CD, kind="ExternalInput").ap()
    w_d = nc.dram_tensor("w", [C, 576], CD, kind="ExternalInput").ap()
    b_d = nc.dram_tensor("b", [128, 5], F32, kind="ExternalInput").ap()
    out_d = nc.dram_tensor("out", [HPC * HD, T], F32, kind="ExternalOutput").ap()

    # head slot map: (q_tile, q_lo, k_tile, k_lo, v_tile, v_lo)
    # tile0=[Q0;Q1] tile1=[K0;K1] tile2=[Q2;V0] tile3=[K2;V1] tile4=[V2]
    SLOT = [
        (0, 0, 1, 0, 2, 64),
        (0, 64, 1, 64, 3, 64),
        (2, 0, 3, 0, 4, 0),
    ]

    with tile.TileContext(nc) as tc:
        with (
            tc.tile_pool(name="const", bufs=1) as cpool,
            tc.tile_pool(name="persist", bufs=1) as sb,
            tc.tile_pool(name="wsb", bufs=1) as wpool,
            tc.tile_pool(name="xn", bufs=12) as xpool,
            tc.tile_pool(name="ps_lo", bufs=1, space="PSUM") as pslo,
            tc.tile_pool(name="ps_hi", bufs=1, space="PSUM") as pshi,
            tc.tile_pool(name="pT", bufs=3) as ptp,
            tc.tile_pool(name="nrm", bufs=2) as nrm,
        ):
            bias_sb = cpool.tile([128, 5], F32)
            nc.sync.dma_start(bias_sb[:], b_d[:])

            qkv_sb = [sb.tile([128, T], CD, name=f"qkv{m}") for m in range(4)]
            qkv_sb.append(sb.tile([64, T], CD, name="qkv4"))
            # duplicates of h2's Q^T/K^T at partitions 64-127 (for the HI stream)
            qdup = sb.tile([128, T], CD, name="qdup")
            kdup = sb.tile([128, T], CD, name="kdup")
            v_aug = [sb.tile([128, NKC * VST], CD, name=f"vaug{h}")
                     for h in range(HPC)]
            # col 64 of every VST block must be 1.0 (denominator ones row);
            # cols 0-63 get overwritten by the V^T xbar transposes.
            v_aug3 = []
            for h in range(HPC):
                nc.vector.memset(v_aug[h][:], 1.0)
                v_aug3.append(v_aug[h][:].rearrange("p (n s) -> p n s", s=VST))

            w_sb = [wpool.tile([128, 576], CD, name=f"w{k}") for k in range(6)]
            for k in range(6):
                nc.sync.dma_start(w_sb[k][:], w_d[128 * k:128 * (k + 1), :])

            # ---------- projection generator (time-shares stream PSUM tags) --
            def proj_mms(pj, col, m):
                mw = 128 if m < 4 else 64
                for k in range(6):
                    nc.tensor.matmul(
                        pj[:mw, 512 * col:512 * col + 512],
                        lhsT=w_sb[k][:, 128 * m:128 * m + mw],
                        rhs=xn_cur[k][:], start=(k == 0), stop=(k == 5))

            def proj_scatter(stg, col, n, m):
                # bias-add + scatter from bf16 staging (off the PSUM chain)
                mw = 128 if m < 4 else 64
                nc.vector.tensor_scalar(
                    out=qkv_sb[m][:mw, 512 * n:512 * (n + 1)],
                    in0=stg[:mw, 512 * col:512 * col + 512],
                    scalar1=bias_sb[:mw, m:m + 1], scalar2=None, op0=AL.add)
                if m >= 2:
                    h = m - 2
                    vt, vlo = SLOT[h][4], SLOT[h][5]
                    nc.sync.dma_start_transpose(
                        v_aug3[h][:, 4 * n:4 * n + 4, 0:64],
                        qkv_sb[vt][vlo:vlo + 64, 512 * n:512 * (n + 1)])
                    if h == 0:
                        nc.sync.dma_start(
                            qdup[64:128, 512 * n:512 * (n + 1)],
                            qkv_sb[2][0:64, 512 * n:512 * (n + 1)])
                    elif h == 1:
                        nc.sync.dma_start(
                            kdup[64:128, 512 * n:512 * (n + 1)],
                            qkv_sb[3][0:64, 512 * n:512 * (n + 1)])

            xn_cur = None

            def proj_gen():
                nonlocal xn_cur
                for n in range(NQ):
                    xn_cur = []
                    for k in range(6):
                        t = xpool.tile([128, 512], CD, tag="xn",
                                       name=f"xn{n}_{k}")
                        nc.sync.dma_start(
                            t[:],
                            xT_d[128 * k:128 * (k + 1), 512 * n:512 * (n + 1)])
                        xn_cur.append(t)
                    pjA = pslo.tile([128, GRP * 512], F32, tag="pslo",
                                    name=f"pjA{n}", bufs=1)
                    for m in range(3):
                        proj_mms(pjA, m, m)
                    stgA = xpool.tile([128, GRP * 512], CD, tag="stg",
                                      name=f"stgA{n}", bufs=2)
                    nc.vector.tensor_copy(stgA[:], pjA[:])
                    for m in range(3):
                        proj_scatter(stgA, m, n, m)
                    yield ('mid', n)
                    pjB = pshi.tile([128, GRP * 512], F32, tag="pshi",
                                    name=f"pjB{n}", bufs=1)
                    for m in range(3, 5):
                        proj_mms(pjB, m - 3, m)
                    stgB = xpool.tile([128, GRP * 512], CD, tag="stg",
                                      name=f"stgB{n}", bufs=2)
                    nc.vector.tensor_copy(stgB[:, 0:1024], pjB[:, 0:1024])
                    for m in range(3, 5):
                        proj_scatter(stgB, m - 3, n, m)
                    yield ('done', n)

            # ---------- attention machinery ----------
            class SState:
                def __init__(self, side):
                    self.side = side
                    self.slot = 0
                    self.batch_id = 0
                    self.den = None
                    self.rcp_by_batch = {}
                    self.finishq = deque()

            def flush_recip(st):
                if st.slot == 0:
                    return
                rcp = nrm.tile([4, 512], F32, tag=f"rcp{st.side}",
                               name=f"rcp_{st.side}_{st.batch_id}", bufs=2)
                if RECIP_FAST:
                    nc.vector.reciprocal_approx_fast(
                        out=rcp[0:st.slot, :], in_=st.den[0:st.slot, :])
                else:
                    nc.vector.reciprocal(rcp[0:st.slot, :],
                                         st.den[0:st.slot, :])
                st.rcp_by_batch[st.batch_id] = rcp
                st.batch_id += 1
                st.slot = 0
                st.den = None

            def maybe_finish(st, limit):
                done = 0
                while st.finishq and done < limit:
                    h, J, uo, bid, slot = st.finishq[0]
                    if bid not in st.rcp_by_batch:
                        break
                    st.finishq.popleft()
                    rcp = st.rcp_by_batch[bid]
                    rcp0 = nrm.tile([1, 512], F32, tag=f"rcp0{st.side}",
                                    name=f"rc0_{st.side}_{h}_{J}", bufs=2)
                    nc.sync.dma_start(rcp0[:], rcp[slot:slot + 1, :])
                    rb = nrm.tile([64, 512], F32, tag=f"rb{st.side}",
                                  name=f"rb_{st.side}_{h}_{J}", bufs=2)
                    nc.gpsimd.partition_broadcast(rb[:], rcp0[:])
                    otn = nrm.tile([64, 512], F32, tag=f"otn{st.side}",
                                   name=f"ot_{st.side}_{h}_{J}", bufs=2)
                    nc.vector.tensor_tensor(out=otn[:], in0=uo[0:64, :],
                                            in1=rb[:], op=AL.mult)
                    nc.sync.dma_start(
                        out_d[HD * h:HD * (h + 1), 512 * J:512 * (J + 1)],
                        otn[:])
                    done += 1

            def emit_pv(pend, po, h, J):
                pg0, pg1, ppT = pend
                nK = 4 * (J + 1)
                for kc in range(pg0, pg1):
                    jj = kc - pg0
                    nc.tensor.matmul(
                        po[:], lhsT=v_aug[h][:, VST * kc:VST * kc + 65],
                        rhs=ppT[:, 512 * jj:512 * (jj + 1)],
                        start=(kc == 0), stop=(kc == nK - 1))
                    yield

            def unit(side, h, J, qs_ap, k_tile, k_lo, pspool, potag, st):
                nK = 4 * (J + 1)
                po = pspool.tile([65, 512], F32, tag=potag,
                                 name=f"po_{side}_{h}_{J}", bufs=1)
                pending = None
                for g0 in range(0, nK, GRP):
                    g1 = min(g0 + GRP, nK)
                    wid = 512 * (g1 - g0)
                    ps_s = pspool.tile([128, GRP * 512], F32,
                                       tag=f"ps{side}",
                                       name=f"ps_{side}_{h}_{J}_{g0}",
                                       bufs=1)
                    for kc in range(g0, g1):
                        jj = kc - g0
                        d = kc - 4 * J
                        c0 = 128 * d if (S_TRIM and d > 0) else 0
                        nc.tensor.matmul(
                            ps_s[:, 512 * jj + c0:512 * (jj + 1)],
                            lhsT=k_tile[k_lo:k_lo + 64,
                                        128 * kc:128 * (kc + 1)],
                            rhs=qs_ap[:, c0:512], start=True, stop=True)
                        yield
                    pT = ptp.tile([128, GRP * 512], CD, tag=f"pT{side}",
                                  name=f"pT_{side}_{h}_{J}_{g0}", bufs=3)
                    nc.scalar.activation(pT[:, :wid], ps_s[:, :wid],
                                         AF.Exp, scale=0.125)
                    for kc in range(g0, g1):
                        d = kc - 4 * J
                        if d >= 0:
                            jj = kc - g0
                            nc.gpsimd.affine_select(
                                out=pT[:, 512 * jj:512 * (jj + 1)],
                                in_=pT[:, 512 * jj:512 * (jj + 1)],
                                compare_op=AL.is_ge, fill=0.0,
                                base=-128 * d, channel_multiplier=-1,
                                pattern=[[1, 512]])
                    # paced PE activity during the ACT window: a throwaway
                    # matmul reading pT (post-exp/mask) and scribbling on the
                    # already-consumed ps_s buffer. It executes right as EXP
                    # finishes, keeping the HAM activity monitor from
                    # re-throttling the PE clock during ACT-bound stretches.
                    # The scribbled region is either rewritten (start=True) by
                    # the next group's S matmuls or lands in the causally
                    # masked zone that affine_select zeroes post-exp.
                    for _ in range(KEEPWARM):
                        nc.tensor.matmul(
                            ps_s[0:64, 0:128], lhsT=pT[0:64, 0:64],
                            rhs=pT[0:64, 0:128], start=True, stop=True)
                    if pending is not None:
                        yield from emit_pv(pending, po, h, J)
                    pending = (g0, g1, pT)
                    yield
                yield from emit_pv(pending, po, h, J)
                # normalize prologue: free the po bank quickly
                # (rows 0-63 = unnormalized out, row 64 = denominator)
                uo = nrm.tile([65, 512], F32, tag=f"uo{side}",
                              name=f"uo_{side}_{h}_{J}", bufs=6)
                nc.vector.tensor_copy(uo[:], po[:])
                if st.slot == 0:
                    st.den = nrm.tile([4, 512], F32, tag=f"den{side}",
                                      name=f"den_{side}_{st.batch_id}",
                                      bufs=2)
                nc.sync.dma_start(st.den[st.slot:st.slot + 1, :],
                                  uo[64:65, :])
                st.finishq.append((h, J, uo, st.batch_id, st.slot))
                st.slot += 1
                if st.slot == 4:
                    flush_recip(st)
                maybe_finish(st, limit=1)

            def stream(side, units, pspool, potag, st):
                for h, J, qs_ap, k_tile, k_lo in units:
                    # h0 deps (Q0/K0/V0-transpose, m<=2) are all in step J's
                    # A-half; h1/h2 deps (K2/kdup/v_aug[1,2]) land in the
                    # B-half, so they must gate on the fully-emitted step.
                    yield ('gate', 2 * J if h == 0 else 2 * J + 1)
                    yield from unit(side, h, J, qs_ap, k_tile, k_lo,
                                    pspool, potag, st)
                flush_recip(st)
                maybe_finish(st, limit=len(st.finishq))

            def qs(tile_idx, lo, J):
                return qkv_sb[tile_idx][lo:lo + 64, 512 * J:512 * (J + 1)]

            h2_lo = (3, 5, 7)
            h2_hi = tuple(J for J in range(NQ) if J not in h2_lo)
            lo_units = [(0, J, qs(0, 0, J), qkv_sb[1], 0) for J in range(NQ)]
            lo_units += [(2, J, qs(2, 0, J), qkv_sb[3], 0) for J in h2_lo]
            hi_units = [(1, J, qs(0, 64, J), qkv_sb[1], 64) for J in range(NQ)]
            hi_units += [(2, J, qdup[64:128, 512 * J:512 * (J + 1)], kdup, 64)
                         for J in h2_hi]
            lo_units.sort(key=lambda u: (u[1], u[0]))
            hi_units.sort(key=lambda u: (u[1], u[0]))

            class Pumped:
                def __init__(self, gen):
                    self.gen = gen
                    self.parked = None
                    self.alive = True

            streams = [
                Pumped(stream("lo", lo_units, pslo, "polo", SState("lo"))),
                Pumped(stream("hi", hi_units, pshi, "pohi", SState("hi"))),
            ]

            def pump(allowed, max_ops):
                ops = 0
                while ops < max_ops:
                    progress = False
                    for s in streams:
                        if not s.alive:
                            continue
                        if s.parked is not None and s.parked > allowed:
                            continue
                        s.parked = None
                        try:
                            y = next(s.gen)
                        except StopIteration:
                            s.alive = False
                            continue
                        if isinstance(y, tuple) and y[0] == 'gate':
                            s.parked = y[1]
                            if s.parked <= allowed:
                                s.parked = None
                                progress = True
                            continue
                        ops += 1
                        progress = True
                    if not progress:
                        break

            # gate only on fully-emitted projection steps: Tile dependency
            # tracking is emission-order-based, so consumers must be emitted
            # after their producers (e.g. kdup chunk DMAs land in the B half).
            allowed = -1
            for kind, n in proj_gen():
                allowed = 2 * n if kind == 'mid' else 2 * n + 1
                pump(allowed, PUMP_OPS)
            pump(10 ** 9, 10 ** 9)

    nc.compile()
    _CACHE['nc'] = nc
    return nc


def _prep_inputs(x, w_qkv, b_qkv):
    """Host-side sharding: per-core xT, column-reordered weight stack, bias."""
    import ml_dtypes
    cdt = ml_dtypes.bfloat16
    x = np.asarray(x, dtype=np.float32)
    w_qkv = np.asarray(w_qkv, dtype=np.float32)
    b_qkv = np.asarray(b_qkv, dtype=np.float32)
    xTs = [np.ascontiguousarray(x[b].T).astype(cdt) for b in range(B)]
    in_maps = []
    for c in range(NCORES):
        b_idx, g = c // 4, c % 4
        H = [3 * g, 3 * g + 1, 3 * g + 2]
        q = lambda h: np.arange(64 * h, 64 * (h + 1))
        k = lambda h: np.arange(C + 64 * h, C + 64 * (h + 1))
        v = lambda h: np.arange(2 * C + 64 * h, 2 * C + 64 * (h + 1))
        cols = np.concatenate([
            q(H[0]), q(H[1]),
            k(H[0]), k(H[1]),
            q(H[2]), v(H[0]),
            k(H[2]), v(H[1]),
            v(H[2]),
        ])
        w_stack = np.ascontiguousarray(w_qkv[:, cols]).astype(cdt)
        b_stack = b_qkv[cols]
        bias_pad = np.zeros((128, 5), dtype=np.float32)
        for m in range(4):
            bias_pad[:, m] = b_stack[128 * m:128 * (m + 1)]
        bias_pad[:64, 4] = b_stack[512:576]
        in_maps.append({"xT": xTs[b_idx], "w": w_stack, "b": bias_pad})
    return in_maps


def _run(x, w_qkv, b_qkv, n_head, **run_kwargs):
    assert int(n_head) == NH and x.shape == (B, T, C)
    nc = _build()
    in_maps = _prep_inputs(x, w_qkv, b_qkv)
    res = bass_utils.run_bass_kernel_spmd(
        nc, in_maps, core_ids=list(range(NCORES)), **run_kwargs)
    out = np.empty((B, T, C), dtype=np.float32)
    for c in range(NCORES):
        b_idx, g = c // 4, c % 4
        out[b_idx, :, 192 * g:192 * (g + 1)] = res.results[c]["out"].T
    return out, res


def kernel(x, w_qkv, b_qkv, n_head):
    return _run(x, w_qkv, b_qkv, n_head)[0]
